# revision 101
# baseline (speedup 1.0000x reference)
"""SAGAN-style self-attention block on 8 Trainium2 NeuronCores (fp8 version).

Reference computation (per batch image, B=8, H=W=64, C=256, Cq=32):
    xf = x.reshape(N=4096, C)
    f = xf @ Wf + bf; g = xf @ Wg + bg; h = xf @ Wh + bh
    s = g @ f.T                  # [N, N]
    beta = softmax(s, axis=-1)
    o = beta @ h
    out = gamma * o + xf

Sharding: data-parallel over batch, one image per NeuronCore (8 cores),
no collectives.

Per-core kernel design:
  - Projections f/g/h run in bf16 (fp32 PSUM accumulation); f, g, h are
    stored in fp8e4m3 for the two big matmuls. g is pre-scaled by
    A_SCH = 4/ln2 so the fp8 exp bit-trick below needs no separate affine.
  - The score matmul s^T = f_aug^T @ g_aug and the output matmul o = e^T @ h
    both run in fp8 DoubleRow perf mode (two contraction slabs per
    instruction at 0.5 PE cycles per output column). The 33-row contraction
    (32 channels + 1 aug row) is split into 17+16 row slabs.
  - A per-query shift is folded into the score matmul via the augmented
    contraction row: f_aug row = 2.0 paired with g_aug row =
    (-A_SCH*M~_n + C_SCH)/2. Any consistent per-query shift divides out of
    softmax, so M~ only has to place each row's scores inside fp8e5m2's
    representable window. M~_n = max(samplemax_n, min(sig_n, samplemax_n
    + SIG_CAP)) + M_MARGIN, where samplemax is an exact row-max over the
    first 256 keys (computed by natural-orientation DR matmuls + DVE
    reduces) and sig_n = 4.078 * |g_n| * sqrt(tr(F^T F)/(32 N)) is an
    isotropic row-max estimate.
  - exp is a saturating approximation (cannot overflow -> no NaN/Inf):
      * ACT half: Sigmoid((s'' - C_SCH)/A_SCH) -> e5m2. Equals exp within
        ~2% for arguments <= -2.5 and saturates to 1 above.
      * DVE half: clamp(round(s''), 0, C_SCH) written as int8 and
        bit-cast to e5m2 (Schraudolph exponent trick, also capped at ~1).
    Each 512-query block runs 16 key-chunk-pair tiles; each tile's two
    512-col halves go to ACT and DVE concurrently from four single-bank
    PSUM score tiles (GPSIMD cannot read PSUM, so Pool only gets
    SBUF-side work: the residual add of the finalize, weight casts, and
    parts of the M~ chain).
  - o-matmuls trail the score/exp stream by LAG pair-tiles so the
    in-order PE never waits on a fresh exp result.
  - Row-sums come for free from a ones-column appended to h (h_aug[:, :C]
    already folds gamma and bias_h). The finalize adds EPS_ROWSUM before
    the reciprocal so fully-flushed rows degrade to the residual instead
    of NaN.
  - The residual add uses the original fp32 x, so for gamma == 0 the
    output is bit-exact x.
"""

import os
from contextlib import ExitStack

import numpy as np

import concourse.bass as bass
import concourse.tile as tile
from concourse import bacc, mybir
from concourse import bass_utils

N_CORES = 8
B, HH, WW, C = 8, 64, 64, 256
N = HH * WW        # 4096 pixels
CQ = C // 8        # 32
NCH = N // 128     # 32 chunks of 128 pixels
NB = N // 512      # 8 blocks of 512 score columns
HAUG = C + 1       # 257: h plus ones column

F32 = mybir.dt.float32
BF16 = mybir.dt.bfloat16
E4 = mybir.dt.float8e4
E5 = mybir.dt.float8e5
I8 = mybir.dt.int8
DR = mybir.MatmulPerfMode.DoubleRow
AF = mybir.ActivationFunctionType
ALU = mybir.AluOpType

LN2 = float(np.log(2.0))
A_SCH = 4.0 / LN2            # e5m2 exponent scale; g2 is pre-scaled by this
C_SCH = 60.0 - 0.25          # schraudolph constant (bias 15*4, tuned)
F_AUG = 2.0                  # f aug-row value (halves the g aug row range)
SIG_SCALE2 = (4.078 ** 2) / 32.0   # (sqrt(2 ln N))^2 / CQ, folded w/ tr(FTF)
SIG_CAP = 8.0                # cap sigma-estimate at samplemax + this
M_MARGIN = 3.0               # extra shift margin (score units)
EPS_ROWSUM = 1e-9

# exp engine pair per key-chunk-pair tile (16 per block): each tile's two
# 512-col halves go to two DIFFERENT engines so they run concurrently and
# the s-psum frees after ~one half-instruction latency.
# A=ACT sigmoid, D=DVE schraudolph-clamp
PATT = ["AD"] * 16


def _bcast_ap(dram_ap, parts, free):
    return bass.AP(
        tensor=dram_ap.tensor,
        offset=dram_ap.offset,
        ap=[[0, parts], [1, free]],
    )


def _col_ap(dram_ap, n):
    """[n,1] column AP over a 1-D DRAM tensor."""
    return bass.AP(tensor=dram_ap.tensor, offset=dram_ap.offset,
                   ap=[[1, n], [0, 1]])


def _emit(ctx: ExitStack, tc: tile.TileContext, io: dict):
    nc = tc.nc
    xb, wf, wg, wh, bf, bg, bh, gamma, ob = (
        io["xb"], io["wf"], io["wg"], io["wh"],
        io["bf"], io["bg"], io["bh"], io["gamma"], io["ob"],
    )
    x16 = io["x16"]

    const = ctx.enter_context(tc.tile_pool(name="const", bufs=1))
    big = ctx.enter_context(tc.tile_pool(name="big", bufs=1))
    epool = ctx.enter_context(tc.tile_pool(name="epool", bufs=10))
    fin = ctx.enter_context(tc.tile_pool(name="fin", bufs=16))
    outp = ctx.enter_context(tc.tile_pool(name="outp", bufs=6))

    # ---- constants / weights ----------------------------------------------
    # preload the ACT sigmoid (+sqrt) tables while DMAs run
    junk = const.tile([128, 8], F32, tag="junk")
    nc.vector.memset(junk[:], 0.0)
    nc.scalar.activation(junk[:], junk[:], AF.Sigmoid)
    nc.scalar.activation(junk[:], junk[:], AF.Sqrt)
    nc.scalar.activation(junk[:], junk[:], AF.Square)

    # x in bf16 (host-cast input): natural-layout load for PE transposes;
    # half the bytes of the fp32 copy and bf16 transposes run at 1 cyc/row
    xn16 = big.tile([128, NCH * C], BF16, tag="xn16")
    xn16_3d = xn16[:].rearrange("p (i c) -> p i c", c=C)
    x16_3d = x16.rearrange("(i p) c -> p i c", p=128)
    nc.sync.dma_start(xn16_3d[:, 0:2, :], x16_3d[:, 0:2, :])
    nc.sync.dma_start(xn16_3d[:, 2:8, :], x16_3d[:, 2:8, :])
    nc.sync.dma_start(xn16_3d[:, 8:20, :], x16_3d[:, 8:20, :])
    nc.sync.dma_start(xn16_3d[:, 20:32, :], x16_3d[:, 20:32, :])

    xbT_bf = big.tile([128, 2 * N], BF16, tag="xbT_bf")          # [p,(k,n)]
    xbT_e4 = big.tile([128, 2 * N], E4, tag="xbT_e4")            # DR slabs
    xbT_bf3 = xbT_bf[:].rearrange("p (k n) -> p k n", k=2)
    xbT_e43 = xbT_e4[:].rearrange("p (k n) -> p k n", k=2)

    # weight DMAs (tiny transfers; the wfg/wh fp8 builds gate the
    # first projection)
    wf_f = const.tile([128, 2 * CQ], F32, tag="wf_f")
    wg_f = const.tile([128, 2 * CQ], F32, tag="wg_f")
    wh_f = const.tile([128, 2 * C], F32, tag="wh_f")
    nc.sync.dma_start(wf_f[:].rearrange("p (k j) -> p k j", k=2),
                      wf.rearrange("(k p) j -> p k j", p=128))
    nc.sync.dma_start(wg_f[:].rearrange("p (k j) -> p k j", k=2),
                      wg.rearrange("(k p) j -> p k j", p=128))
    nc.sync.dma_start(wh_f[:].rearrange("p (k c) -> p k c", k=2),
                      wh.rearrange("(k p) c -> p k c", p=128))

    # x fp32 buffer (residual path only; loaded after the prologue DMAs so
    # the transposes own the DMA engines early)
    xf_f32 = big.tile([128, NCH * C], F32, tag="xf_f32")
    xf_f32_3d = xf_f32[:].rearrange("p (i c) -> p i c", c=C)
    xb_3d = xb.rearrange("(i p) c -> p i c", p=128)

    # weights fp32 -> fp8 e4m3 DoubleRow slab layout ([p, slab, col]:
    # channel c = slab*128 + p contracts against the matching xbT slab)
    # wfg: f in cols 0:32, g in cols 32:64 (shared-rhs packing: one DR
    # matmul per key group computes f^T and g^T together)
    wfg_e4 = const.tile([128, 2 * 2 * CQ], E4, tag="wfg_e4")
    wfg_3 = wfg_e4[:].rearrange("p (k j) -> p k j", k=2)
    wh_e4 = const.tile([128, 2 * C], E4, tag="wh_e4")
    wh_e4_3 = wh_e4[:].rearrange("p (k c) -> p k c", k=2)
    for k in range(2):
        nc.gpsimd.tensor_copy(wfg_3[:, k, 0:CQ], wf_f[:, k * CQ:(k + 1) * CQ])
        nc.gpsimd.tensor_copy(wfg_3[:, k, CQ:2 * CQ],
                              wg_f[:, k * CQ:(k + 1) * CQ])
    nc.gpsimd.tensor_copy(wh_e4[:], wh_f[:])

    # f/g biases + scales as [64,1] columns matching the packed fg psum:
    # rows 0:32 f ((x@wf)+bf), rows 32:64 g (A_SCH*((x@wg)+bg))
    fgb_col = const.tile([2 * CQ, 1], F32, tag="fgb_col")
    nc.sync.dma_start(fgb_col[0:CQ, :], _col_ap(bf, CQ))
    nc.sync.dma_start(fgb_col[CQ:2 * CQ, :], _col_ap(bg, CQ))
    nc.gpsimd.tensor_scalar(fgb_col[CQ:2 * CQ, :], fgb_col[CQ:2 * CQ, :],
                            A_SCH, None, ALU.mult)
    fgs_col = const.tile([2 * CQ, 1], F32, tag="fgs_col")
    nc.vector.memset(fgs_col[0:CQ, :], 1.0)
    nc.vector.memset(fgs_col[CQ:2 * CQ, :], A_SCH)
    # bh enters each h psum through a trailing 1-row bf16 matmul (ones
    # lhsT x bh rhs) so no per-chunk vector add is needed; the gamma
    # scale in the copy-out covers it too
    bh_row = const.tile([1, C], F32, tag="bh_row")
    nc.sync.dma_start(bh_row[:], bass.AP(tensor=bh.tensor, offset=bh.offset,
                                         ap=[[0, 1], [1, C]]))
    bh_row_b = const.tile([1, C], BF16, tag="bh_row_b")
    nc.vector.tensor_copy(bh_row_b[:], bh_row[:])
    ones1_b = const.tile([1, 128], BF16, tag="ones1_b")
    nc.vector.memset(ones1_b[:], 1.0)
    gamma_bc = const.tile([128, 1], F32, tag="gamma_bc")
    nc.sync.dma_start(gamma_bc[:], _bcast_ap(gamma, 128, 1))

    ident_f = const.tile([128, 128], F32, tag="ident_f")
    from concourse.masks import make_identity
    make_identity(nc, ident_f[:])
    ident_b = const.tile([128, 128], BF16, tag="ident_b")
    nc.gpsimd.tensor_copy(ident_b[:], ident_f[:])

    # small consts
    two_e4 = const.tile([1, 128], E4, tag="two_e4")
    nc.vector.memset(two_e4[:], F_AUG)
    zero_e4 = const.tile([1, 128], E4, tag="zero_e4")
    nc.vector.memset(zero_e4[:], 0.0)
    sigb_bc = const.tile([128, 1], F32, tag="sigb_bc")
    nc.vector.memset(sigb_bc[:], -C_SCH / A_SCH)

    def bcast_sb(src_tile, parts, free):
        """AP replicating src_tile[0:1, ...] across partitions (and cols)."""
        ap = src_tile[0:1, 0:1]
        pstep = 1 if parts == 1 else 0
        if free <= 128:
            shape = [[pstep, parts], [1, free]]
        else:
            shape = [[pstep, parts], [0, free // 128], [1, 128]]
        return bass.AP(tensor=ap.tensor, offset=ap.offset, ap=shape)

    # ---- big SBUF tensors --------------------------------------------------
    f2 = big.tile([17, NCH * 2 * 128], E4, tag="f2")             # [p,(m,sl,c)]
    g2 = big.tile([17, 2 * N], E4, tag="g2")                     # [p,(sl,n)]
    stage_fg = big.tile([2 * CQ, N], E4, tag="stage_fg")         # f:0-31 g:32-63
    h_aug = big.tile([128, NCH * HAUG], E4, tag="h_aug")
    g_nat = big.tile([128, NCH * CQ], BF16, tag="g_nat")
    gsqn = big.tile([128, NCH * CQ], BF16, tag="gsqn")
    fsq_s = big.tile([32, 512], BF16, tag="fsq_s")               # scratch
    facc = big.tile([32, 8], F32, tag="facc")
    m_col = big.tile([128, NCH], BF16, tag="m_col")
    gn_col = big.tile([128, NCH], F32, tag="gn_col")
    mneg = big.tile([128, NCH], BF16, tag="mneg")
    fs_smp = big.tile([2 * CQ, 128], E4, tag="fs_smp")
    trv8 = big.tile([1, 8], F32, tag="trv8")
    trv_bc = big.tile([128, 1], F32, tag="trv_bc")
    scale_bc = big.tile([128, 1], F32, tag="scale_bc")
    sig_col = big.tile([128, NCH], F32, tag="sig_col")
    t2_col = big.tile([128, NCH], F32, tag="t2_col")
    t1_col = big.tile([128, NCH], F32, tag="t1_col")
    t3_col = big.tile([128, NCH], F32, tag="t3_col")

    f2_4d = f2[:].rearrange("p (m sl c) -> p m sl c", sl=2, c=128)
    g2_3d = g2[:].rearrange("p (sl n) -> p sl n", sl=2)
    h_aug_3d = h_aug[:].rearrange("p (m c) -> p m c", c=HAUG)
    g_nat_3d = g_nat[:].rearrange("p (m c) -> p m c", c=CQ)

    # round-robin engine dispatch for copy-out distribution. GPSIMD cannot
    # read PSUM, so drains rotate between ACT and DVE only.
    _engines = [nc.scalar, nc.vector]
    _rr = [0]

    def rr_copy(out, in_):
        e = _engines[_rr[0] % 2]
        _rr[0] += 1
        if e is nc.scalar:
            nc.scalar.copy(out, in_)
        else:
            e.tensor_copy(out, in_)

    def rr_add(out, in_, bias_ap):
        e = _engines[_rr[0] % 2]
        _rr[0] += 1
        if e is nc.scalar:
            nc.scalar.activation(out, in_, AF.Identity, bias=bias_ap)
        else:
            e.tensor_scalar(out, in_, bias_ap, None, ALU.add)

    def rr_add_scale(out, in_, bias_ap, scaled_bias_ap, scale):
        """out = (in_ + bias) * scale."""
        e = _engines[_rr[0] % 2]
        _rr[0] += 1
        if e is nc.scalar:
            nc.scalar.activation(out, in_, AF.Identity, scale=scale,
                                 bias=scaled_bias_ap)
        else:
            e.tensor_scalar(out, in_, bias_ap, scale, ALU.add, ALU.mult)

    # ---- prologue: transposes, projections, sample-max (interleaved) ------
    # aug rows first (consts only): f2 slab0 row16 = F_AUG, slab1 row16 = 0;
    # g2 row16 = 0 in both slabs (slab0 is read as 0 by the sample-max
    # matmuls, then overwritten with the -M~ row)
    nc.sync.dma_start(f2_4d[16:17, :, 0, :], bcast_sb(two_e4, 1, N))
    nc.sync.dma_start(f2_4d[16:17, :, 1, :], bcast_sb(zero_e4, 1, N))
    nc.sync.dma_start(g2_3d[16:17, :, :], bcast_sb(zero_e4, 1, 2 * N))

    with tc.tile_pool(name="ps_t", bufs=2, space="PSUM") as ps_t, \
         tc.tile_pool(name="ps_w", bufs=5, space="PSUM") as ps_w, \
         tc.tile_pool(name="ps_sub", bufs=1, space="PSUM") as ps_sub:

        def emit_xT(mt):
            """Group mt: PE-transpose 8 [128,128] bf16 blocks, drain ->
            xbT bf16, one DVE 2x cast -> e4 DR slabs. The first groups'
            drains go to ACT, which is otherwise idle in the head while
            DVE is the early bottleneck."""
            for k in range(2):
                tp = ps_t.tile([128, 512], BF16, tag="tp", name=f"tp{mt}_{k}")
                for idx, i in enumerate(range(mt * 4, mt * 4 + 4)):
                    nc.tensor.transpose(
                        tp[:, idx * 128:(idx + 1) * 128],
                        xn16[:, i * C + k * 128: i * C + k * 128 + 128],
                        ident_b[:])
                if mt < 4:
                    nc.scalar.copy(xbT_bf3[:, k, mt * 512:(mt + 1) * 512],
                                   tp[:])
                else:
                    nc.vector.tensor_copy(
                        xbT_bf3[:, k, mt * 512:(mt + 1) * 512], tp[:])
            cast_eng = nc.gpsimd if mt < 4 else nc.vector
            cast_eng.tensor_copy(
                xbT_e43[:, :, mt * 512:(mt + 1) * 512],
                xbT_bf3[:, :, mt * 512:(mt + 1) * 512])

        emit_xT(0)
        emit_xT(1)

        def emit_mops(c0, c1):
            sl = slice(c0, c1)
            nc.scalar.activation(sig_col[:, sl], gn_col[:, sl], AF.Sqrt,
                                 scale=scale_bc[:])
            nc.gpsimd.tensor_scalar(t1_col[:, sl], m_col[:, sl],
                                    SIG_CAP * A_SCH, None, ALU.add)
            nc.vector.tensor_tensor(t2_col[:, sl], sig_col[:, sl],
                                    t1_col[:, sl], op=ALU.min)
            nc.vector.tensor_tensor(t3_col[:, sl], m_col[:, sl],
                                    t2_col[:, sl], op=ALU.max)
            nc.gpsimd.tensor_scalar(mneg[:, sl], t3_col[:, sl],
                                    -1.0 / F_AUG,
                                    (C_SCH - M_MARGIN * A_SCH) / F_AUG,
                                    ALU.mult, ALU.add)

        def emit_mfold(c0, c1):
            w = c1 - c0
            ps_mt = ps_w.tile([w, 128], BF16, tag="w", name=f"psmt{c0}")
            nc.tensor.transpose(ps_mt[:], mneg[:, c0:c1], ident_b[:])
            mst = fin.tile([32, 128], E4, tag="mst", name=f"mst{c0}")
            nc.vector.tensor_copy(mst[0:w, :], ps_mt[:])
            nc.sync.dma_start(g2_3d[16:17, 0, c0 * 128:c1 * 128], mst[0:w, :])

        for mt in range(8):
            if mt + 2 < 8:
                emit_xT(mt + 2)
            # packed f^T|g^T: one DR matmul per group ([64, 512] psum)
            ps_fg = ps_w.tile([2 * CQ, 512], F32, tag="w", name=f"psfg{mt}")
            nc.tensor.matmul(ps_fg[:], lhsT=wfg_3[:, :, :],
                             rhs=xbT_e43[:, :, mt * 512:(mt + 1) * 512],
                             start=True, stop=True, perf_mode=DR)

            # copy-out with per-row bias+scale (f rows: +bf, g rows:
            # A_SCH*(g+bg)) into staging; DMAs below remap to slab layout
            nc.scalar.activation(stage_fg[:, mt * 512:(mt + 1) * 512],
                                 ps_fg[:], AF.Identity, scale=fgs_col[:],
                                 bias=fgb_col[:])

            # |f|^2 accumulation for tr(F^T F) (group 0 sample is enough
            # for this global scale estimate)
            if mt == 0:
                nc.scalar.activation(fsq_s[:], ps_fg[0:CQ, :], AF.Square,
                                     accum_out=facc[:, 0:1])
                # f sample columns duplicated at partitions 32:64 so the
                # natural-orientation sample matmuls (lhsT = stage g rows)
                # see matching base partitions
                nc.sync.dma_start(fs_smp[CQ:2 * CQ, :],
                                  stage_fg[0:CQ, 0:128])

            # slab remap DMAs once per 2 groups (stage -> f2/g2 layouts)
            if mt % 2 == 1:
                m0 = (mt - 1) * 4
                sl0 = slice((mt - 1) * 512, (mt + 1) * 512)
                nc.sync.dma_start(f2_4d[0:16, m0:m0 + 8, 0, :],
                                  stage_fg[0:16, sl0])
                nc.sync.dma_start(f2_4d[0:16, m0:m0 + 8, 1, :],
                                  stage_fg[16:32, sl0])
                nc.sync.dma_start(g2_3d[0:16, 0, sl0], stage_fg[32:48, sl0])
                nc.sync.dma_start(g2_3d[0:16, 1, sl0], stage_fg[48:64, sl0])

            # g natural (for |g_n|^2): one [128, 128] psum per group
            ps_gn = ps_w.tile([128, 4 * CQ], F32, tag="w", name=f"psgn{mt}")
            for j in range(4):
                m = mt * 4 + j
                nc.tensor.matmul(ps_gn[:, j * CQ:(j + 1) * CQ],
                                 lhsT=xbT_e43[:, :, m * 128:(m + 1) * 128],
                                 rhs=wfg_3[:, :, CQ:2 * CQ],
                                 start=True, stop=True, perf_mode=DR)
            nc.vector.tensor_copy(g_nat_3d[:, mt * 4:(mt + 1) * 4, :], ps_gn[:])

            # h (fp8 DR, + bh via a 1-row bf16 matmul) -> gamma-scaled
            # fp8 h_aug
            for j2 in range(2):
                ps_h = ps_w.tile([128, 2 * C], F32, tag="w",
                                 name=f"psh{mt}_{j2}")
                for jj in range(2):
                    m = mt * 4 + 2 * j2 + jj
                    nc.tensor.matmul(
                        ps_h[:, jj * C:(jj + 1) * C],
                        lhsT=xbT_e43[:, :, m * 128:(m + 1) * 128],
                        rhs=wh_e4_3[:, :, :],
                        start=True, stop=False, perf_mode=DR)
                    nc.tensor.matmul(
                        ps_h[:, jj * C:(jj + 1) * C],
                        lhsT=ones1_b[:], rhs=bh_row_b[:],
                        start=False, stop=True)
                m0 = mt * 4 + 2 * j2
                nc.scalar.activation(h_aug_3d[:, m0:m0 + 2, 0:C], ps_h[:],
                                     AF.Identity, scale=gamma_bc[:])

            # |g_n|^2 incrementally for this group (Pool square + DVE
            # inner-axis reduce) so only the tail remains after group 7
            gsl = slice(mt * 4 * CQ, (mt + 1) * 4 * CQ)
            nc.gpsimd.tensor_tensor(gsqn[:, gsl], g_nat[:, gsl],
                                    g_nat[:, gsl], op=ALU.mult)
            gsq_g = gsqn[:, gsl].rearrange("p (m c) -> p m c", c=CQ)
            nc.vector.tensor_reduce(gn_col[:, mt * 4:(mt + 1) * 4], gsq_g,
                                    mybir.AxisListType.X, ALU.add)

            # tr(F^T F) estimate from the first 7 groups: start the DRAM
            # broadcast roundtrip early so it is off the critical path
            if mt == 0:
                facc_b = big.tile([32, 8], BF16, tag="facc_b")
                nc.gpsimd.tensor_copy(facc_b[:, 0:1], facc[:, 0:1])
                ones32 = const.tile([32, 1], BF16, tag="ones32")
                nc.vector.memset(ones32[:], 1.0)
                ps_tr = ps_w.tile([1, 8], F32, tag="w", name="ps_tr")
                nc.tensor.matmul(ps_tr[:, 0:1], lhsT=ones32[:],
                                 rhs=facc_b[:, 0:1], start=True, stop=True)
                trv1 = big.tile([1, 1], F32, tag="trv1")
                nc.scalar.copy(trv1[:], ps_tr[:, 0:1])
                scr_trv = nc.dram_tensor("scr_trv", [1], F32,
                                         kind="Internal").ap()
                nc.sync.dma_start(scr_trv, trv1[:])
                nc.sync.dma_start(trv_bc[:], _bcast_ap(scr_trv, 128, 1))
                nc.vector.tensor_scalar(scale_bc[:], trv_bc[:],
                                        8.0 * SIG_SCALE2 * A_SCH
                                        * A_SCH / N, None, ALU.mult)



            # sample-max for this group's 4 query chunks, straight off the
            # just-drained stage (natural orientation, non-DR): no wait on
            # the slab-remap DMAs or the aug rows, so the M~ chain runs a
            # full pair earlier. stage g rows are A_SCH-scaled like g2, so
            # m_col stays in the same units.
            for j in range(2):
                qc0 = mt * 4 + 2 * j
                ps_ss = ps_sub.tile([128, 256], F32, tag="ss",
                                    name=f"ss{qc0}")
                for jj in range(2):
                    nc.tensor.matmul(
                        ps_ss[:, jj * 128:(jj + 1) * 128],
                        lhsT=stage_fg[CQ:2 * CQ,
                                      (qc0 + jj) * 128:(qc0 + jj + 1) * 128],
                        rhs=fs_smp[CQ:2 * CQ, :],
                        start=True, stop=True)
                red_in = ps_ss[:].rearrange("p (a c) -> p a c", c=128)
                nc.vector.tensor_reduce(m_col[:, qc0:qc0 + 2], red_in,
                                        mybir.AxisListType.X, ALU.max)
            # M~ chain for the current pair as soon as its gn/sample are
            # done (scale roundtrip is ready from mt~2); pair (0,1) joins
            # at mt==3
            if mt % 2 == 1 and mt >= 3:
                if mt == 3:
                    emit_mops(0, 8)
                    emit_mfold(0, 8)
                emit_mops(4 * (mt - 1), 4 * (mt + 1))
                emit_mfold(4 * (mt - 1), 4 * (mt + 1))

    # h_aug ones column
    nc.vector.memset(h_aug_3d[:, :, C:C + 1], 1.0)

    # ---- main attention loop ----------------------------------------------
    ps_s = ctx.enter_context(tc.tile_pool(name="ps_s", bufs=4, space="PSUM"))
    ps_o = ctx.enter_context(tc.tile_pool(name="ps_o", bufs=4, space="PSUM"))
    ob_3d = ob.rearrange("(k p) c -> p k c", p=128)

    LAG = 4  # o-matmuls trail the s/exp stream by this many pair-tiles
    o_tiles: dict = {}
    e_tiles: dict = {}

    def emit_o(p):
        nb2, t2 = p // 16, p % 16
        o_ps = o_tiles[nb2]
        e_3d = e_tiles.pop(p)[:].rearrange("p (sl n) -> p sl n", sl=2)
        for q in range(4):
            nc.tensor.matmul(
                o_ps[q][:], lhsT=e_3d[:, :, q * 128:(q + 1) * 128],
                rhs=h_aug_3d[:, 2 * t2:2 * t2 + 2, :],
                start=(t2 == 0), stop=(t2 == 15), perf_mode=DR)
        if t2 == 15:
            finalize(nb2)

    def finalize(nb2):
        o_ps = o_tiles.pop(nb2)
        res4 = outp.tile([128, 4 * C], F32, tag="res4", name=f"res4_{nb2}")
        for q in range(4):
            gch = nb2 * 4 + q
            rs = fin.tile([128, 1], F32, tag="rs", name=f"rs{nb2}_{q}")
            nc.vector.tensor_scalar(rs[:], o_ps[q][:, C:C + 1], EPS_ROWSUM,
                                    None, ALU.add)
            recip = fin.tile([128, 1], F32, tag="recip", name=f"rc{nb2}_{q}")
            nc.vector.reciprocal_approx_fast(recip[:], rs[:])
            res_sc = fin.tile([128, C], F32, tag="res_sc",
                              name=f"rsc{nb2}_{q}")
            nc.scalar.activation(res_sc[:], o_ps[q][:, 0:C], AF.Identity,
                                 scale=recip[:])
            nc.gpsimd.tensor_tensor(
                res4[:, q * C:(q + 1) * C], res_sc[:],
                xf_f32[:, gch * C:(gch + 1) * C], op=ALU.add)
        nc.sync.dma_start(
            ob_3d[:, nb2 * 4:(nb2 + 1) * 4, :],
            res4[:].rearrange("p (k c) -> p k c", c=C))

    for p in range(NB * 16):
        nb, t = p // 16, p % 16
        if p == 2:
            # x fp32 for the residual path: DMA engines are idle during
            # the main loop; block-0 finalize consumes chunk 0 much later
            for i4 in range(4):
                nc.sync.dma_start(xf_f32_3d[:, i4 * 8:(i4 + 1) * 8, :],
                                  xb_3d[:, i4 * 8:(i4 + 1) * 8, :])
        if t == 0:
            o_tiles[nb] = [
                ps_o.tile([128, HAUG], F32, tag="o", name=f"o_ps{nb}_{q}")
                for q in range(4)]
        g2_blk = g2_3d[:, :, nb * 512:(nb + 1) * 512]
        e_t = epool.tile([128, 1024], E5, tag="e", name=f"e{nb}_{t}")
        e_tiles[p] = e_t
        # ACT runs ~99% vs DVE ~93% in the main loop: shift one exp half
        # to DVE in every other block to even the pair out
        patt = "DD" if (t == 7 and nb % 2 == 0) else PATT[t]
        for hh, eng_c in enumerate(patt):
            s_ps = ps_s.tile([128, 512], F32, tag="s", name=f"s{nb}_{t}_{hh}")
            nc.tensor.matmul(s_ps[:], lhsT=f2_4d[:, 2 * t + hh, :, :],
                             rhs=g2_blk, start=True, stop=True, perf_mode=DR)
            sl = slice(hh * 512, (hh + 1) * 512)
            if eng_c == "A":
                # s'' holds A_SCH*(s - M~) + C_SCH; undo for the sigmoid
                nc.scalar.activation(e_t[:, sl], s_ps[:], AF.Sigmoid,
                                     scale=1.0 / A_SCH, bias=sigb_bc[:])
            else:
                eng = nc.vector if eng_c == "D" else nc.gpsimd
                eng.tensor_scalar(e_t[:, sl].bitcast(I8), s_ps[:],
                                  0.0, C_SCH, ALU.max, ALU.min)
        if p >= LAG:
            emit_o(p - LAG)
    for p in range(NB * 16 - LAG, NB * 16):
        emit_o(p)


_CACHE: dict = {}


def build():
    if "nc" in _CACHE:
        return _CACHE["nc"]
    nc = bacc.Bacc("TRN2", target_bir_lowering=False, debug=False,
                   num_devices=N_CORES)
    io = {
        "xb": nc.dram_tensor("xb", [N, C], F32, kind="ExternalInput").ap(),
        "x16": nc.dram_tensor("x16", [N, C], BF16, kind="ExternalInput").ap(),
        "wf": nc.dram_tensor("wf", [C, CQ], F32, kind="ExternalInput").ap(),
        "wg": nc.dram_tensor("wg", [C, CQ], F32, kind="ExternalInput").ap(),
        "wh": nc.dram_tensor("wh", [C, C], F32, kind="ExternalInput").ap(),
        "bf": nc.dram_tensor("bf", [CQ], F32, kind="ExternalInput").ap(),
        "bg": nc.dram_tensor("bg", [CQ], F32, kind="ExternalInput").ap(),
        "bh": nc.dram_tensor("bh", [C], F32, kind="ExternalInput").ap(),
        "gamma": nc.dram_tensor("gamma", [1], F32, kind="ExternalInput").ap(),
        "ob": nc.dram_tensor("ob", [N, C], F32, kind="ExternalOutput").ap(),
    }
    with tile.TileContext(nc) as tc:
        with ExitStack() as ctx:
            _emit(ctx, tc, io)
    nc.compile()
    _CACHE["nc"] = nc
    return nc


def kernel(x, kernel_f, kernel_g, kernel_h, bias_f, bias_g, bias_h, gamma):
    import ml_dtypes
    x = np.asarray(x, dtype=np.float32)
    x16 = x.astype(ml_dtypes.bfloat16)
    wf = np.ascontiguousarray(np.asarray(kernel_f, dtype=np.float32))
    wg = np.ascontiguousarray(np.asarray(kernel_g, dtype=np.float32))
    wh = np.ascontiguousarray(np.asarray(kernel_h, dtype=np.float32))
    bf = np.ascontiguousarray(np.asarray(bias_f, dtype=np.float32))
    bg = np.ascontiguousarray(np.asarray(bias_g, dtype=np.float32))
    bh = np.ascontiguousarray(np.asarray(bias_h, dtype=np.float32))
    gm = np.ascontiguousarray(np.asarray(gamma, dtype=np.float32).reshape(1))

    per_core = {
        "xb": [np.ascontiguousarray(x[b].reshape(N, C)) for b in range(N_CORES)],
        "x16": [np.ascontiguousarray(x16[b].reshape(N, C))
                for b in range(N_CORES)],
        "wf": [wf] * N_CORES, "wg": [wg] * N_CORES, "wh": [wh] * N_CORES,
        "bf": [bf] * N_CORES, "bg": [bg] * N_CORES, "bh": [bh] * N_CORES,
        "gamma": [gm] * N_CORES,
    }
    nc = build()
    in_maps = [{nm: per_core[nm][b] for nm in per_core} for b in range(N_CORES)]
    try:
        res = bass_utils.run_bass_kernel_spmd(
            nc, in_maps, core_ids=list(range(N_CORES)))
    except ModuleNotFoundError:
        os.environ["BASS_NEVER_TRACE"] = "1"
        res = bass_utils.run_bass_kernel_spmd(
            nc, in_maps, core_ids=list(range(N_CORES)))
    out = np.stack([res.results[b]["ob"] for b in range(N_CORES)], axis=0)
    return out.reshape(B, HH, WW, C).astype(np.float32)


if __name__ == "__main__":
    rng = np.random.default_rng(0)
    x = rng.standard_normal((B, HH, WW, C)).astype(np.float32)
    lim = np.sqrt(6.0 / (C + CQ))
    out = kernel(
        x,
        rng.uniform(-lim, lim, (C, CQ)).astype(np.float32),
        rng.uniform(-lim, lim, (C, CQ)).astype(np.float32),
        rng.uniform(-lim, lim, (C, C)).astype(np.float32),
        np.zeros(CQ, np.float32), np.zeros(CQ, np.float32),
        np.zeros(C, np.float32), np.zeros(1, np.float32),
    )
    print(out.shape, out.dtype)



# revision 104
# speedup vs baseline: 1.0169x; 1.0169x over previous
"""SAGAN-style self-attention block on 8 Trainium2 NeuronCores (fp8 version).

Reference computation (per batch image, B=8, H=W=64, C=256, Cq=32):
    xf = x.reshape(N=4096, C)
    f = xf @ Wf + bf; g = xf @ Wg + bg; h = xf @ Wh + bh
    s = g @ f.T                  # [N, N]
    beta = softmax(s, axis=-1)
    o = beta @ h
    out = gamma * o + xf

Sharding: data-parallel over batch, one image per NeuronCore (8 cores),
no collectives.

Per-core kernel design:
  - Projections f/g/h run in bf16 (fp32 PSUM accumulation); f, g, h are
    stored in fp8e4m3 for the two big matmuls. g is pre-scaled by
    A_SCH = 4/ln2 so the fp8 exp bit-trick below needs no separate affine.
  - The score matmul s^T = f_aug^T @ g_aug and the output matmul o = e^T @ h
    both run in fp8 DoubleRow perf mode (two contraction slabs per
    instruction at 0.5 PE cycles per output column). The 33-row contraction
    (32 channels + 1 aug row) is split into 17+16 row slabs.
  - A per-query shift is folded into the score matmul via the augmented
    contraction row: f_aug row = 2.0 paired with g_aug row =
    (-A_SCH*M~_n + C_SCH)/2. Any consistent per-query shift divides out of
    softmax, so M~ only has to place each row's scores inside fp8e5m2's
    representable window. M~_n = max(samplemax_n, min(sig_n, samplemax_n
    + SIG_CAP)) + M_MARGIN, where samplemax is an exact row-max over the
    first 256 keys (computed by natural-orientation DR matmuls + DVE
    reduces) and sig_n = 4.078 * |g_n| * sqrt(tr(F^T F)/(32 N)) is an
    isotropic row-max estimate.
  - exp is a saturating approximation (cannot overflow -> no NaN/Inf):
      * ACT half: Sigmoid((s'' - C_SCH)/A_SCH) -> e5m2. Equals exp within
        ~2% for arguments <= -2.5 and saturates to 1 above.
      * DVE half: clamp(round(s''), 0, C_SCH) written as int8 and
        bit-cast to e5m2 (Schraudolph exponent trick, also capped at ~1).
    Each 512-query block runs 16 key-chunk-pair tiles; each tile's two
    512-col halves go to ACT and DVE concurrently from four single-bank
    PSUM score tiles (GPSIMD cannot read PSUM, so Pool only gets
    SBUF-side work: the residual add of the finalize, weight casts, and
    parts of the M~ chain).
  - o-matmuls trail the score/exp stream by LAG pair-tiles so the
    in-order PE never waits on a fresh exp result.
  - Row-sums come for free from a ones-column appended to h (h_aug[:, :C]
    already folds gamma and bias_h). The finalize adds EPS_ROWSUM before
    the reciprocal so fully-flushed rows degrade to the residual instead
    of NaN.
  - The residual add uses the original fp32 x, so for gamma == 0 the
    output is bit-exact x.
"""

import os
from contextlib import ExitStack

import numpy as np

import concourse.bass as bass
import concourse.tile as tile
from concourse import bacc, mybir
from concourse import bass_utils

N_CORES = 8
B, HH, WW, C = 8, 64, 64, 256
N = HH * WW        # 4096 pixels
CQ = C // 8        # 32
NCH = N // 128     # 32 chunks of 128 pixels
NB = N // 512      # 8 blocks of 512 score columns
HAUG = C + 1       # 257: h plus ones column

F32 = mybir.dt.float32
BF16 = mybir.dt.bfloat16
E4 = mybir.dt.float8e4
E5 = mybir.dt.float8e5
I8 = mybir.dt.int8
DR = mybir.MatmulPerfMode.DoubleRow
AF = mybir.ActivationFunctionType
ALU = mybir.AluOpType

LN2 = float(np.log(2.0))
A_SCH = 4.0 / LN2            # e5m2 exponent scale; g2 is pre-scaled by this
C_SCH = 60.0 - 0.25          # schraudolph constant (bias 15*4, tuned)
F_AUG = 2.0                  # f aug-row value (halves the g aug row range)
SIG_SCALE2 = (4.078 ** 2) / 32.0   # (sqrt(2 ln N))^2 / CQ, folded w/ tr(FTF)
SIG_CAP = 8.0                # cap sigma-estimate at samplemax + this
M_MARGIN = 3.0               # extra shift margin (score units)
EPS_ROWSUM = 1e-9

# exp engine pair per key-chunk-pair tile (16 per block): each tile's two
# 512-col halves go to two DIFFERENT engines so they run concurrently and
# the s-psum frees after ~one half-instruction latency.
# A=ACT sigmoid, D=DVE schraudolph-clamp
PATT = ["AD"] * 16


def _bcast_ap(dram_ap, parts, free):
    return bass.AP(
        tensor=dram_ap.tensor,
        offset=dram_ap.offset,
        ap=[[0, parts], [1, free]],
    )


def _col_ap(dram_ap, n):
    """[n,1] column AP over a 1-D DRAM tensor."""
    return bass.AP(tensor=dram_ap.tensor, offset=dram_ap.offset,
                   ap=[[1, n], [0, 1]])


def _emit(ctx: ExitStack, tc: tile.TileContext, io: dict):
    nc = tc.nc
    xb, wf, wg, wh, bf, bg, bh, gamma, ob = (
        io["xb"], io["wf"], io["wg"], io["wh"],
        io["bf"], io["bg"], io["bh"], io["gamma"], io["ob"],
    )
    x16 = io["x16"]

    const = ctx.enter_context(tc.tile_pool(name="const", bufs=1))
    big = ctx.enter_context(tc.tile_pool(name="big", bufs=1))
    epool = ctx.enter_context(tc.tile_pool(name="epool", bufs=10))
    fin = ctx.enter_context(tc.tile_pool(name="fin", bufs=16))
    outp = ctx.enter_context(tc.tile_pool(name="outp", bufs=6))

    # ---- constants / weights ----------------------------------------------
    # preload the ACT sigmoid (+sqrt) tables while DMAs run
    junk = const.tile([128, 8], F32, tag="junk")
    nc.vector.memset(junk[:], 0.0)
    nc.scalar.activation(junk[:], junk[:], AF.Sigmoid)
    nc.scalar.activation(junk[:], junk[:], AF.Sqrt)
    nc.scalar.activation(junk[:], junk[:], AF.Square)

    # x in bf16 (host-cast input): natural-layout load for PE transposes;
    # half the bytes of the fp32 copy and bf16 transposes run at 1 cyc/row
    xn16 = big.tile([128, NCH * C], BF16, tag="xn16")
    xn16_3d = xn16[:].rearrange("p (i c) -> p i c", c=C)
    x16_3d = x16.rearrange("(i p) c -> p i c", p=128)
    nc.sync.dma_start(xn16_3d[:, 0:2, :], x16_3d[:, 0:2, :])
    nc.sync.dma_start(xn16_3d[:, 2:8, :], x16_3d[:, 2:8, :])
    nc.sync.dma_start(xn16_3d[:, 8:20, :], x16_3d[:, 8:20, :])
    nc.sync.dma_start(xn16_3d[:, 20:32, :], x16_3d[:, 20:32, :])

    xbT_e4 = big.tile([128, 2 * N], E4, tag="xbT_e4")            # DR slabs
    xbT_e43 = xbT_e4[:].rearrange("p (k n) -> p k n", k=2)

    # weight DMAs (tiny transfers; the wfg/wh fp8 builds gate the
    # first projection)
    wf_f = const.tile([128, 2 * CQ], F32, tag="wf_f")
    wg_f = const.tile([128, 2 * CQ], F32, tag="wg_f")
    wh_f = const.tile([128, 2 * C], F32, tag="wh_f")
    nc.sync.dma_start(wf_f[:].rearrange("p (k j) -> p k j", k=2),
                      wf.rearrange("(k p) j -> p k j", p=128))
    nc.sync.dma_start(wg_f[:].rearrange("p (k j) -> p k j", k=2),
                      wg.rearrange("(k p) j -> p k j", p=128))
    nc.sync.dma_start(wh_f[:].rearrange("p (k c) -> p k c", k=2),
                      wh.rearrange("(k p) c -> p k c", p=128))

    # x fp32 buffer (residual path only; loaded after the prologue DMAs so
    # the transposes own the DMA engines early)
    xf_f32 = big.tile([128, NCH * C], F32, tag="xf_f32")
    xf_f32_3d = xf_f32[:].rearrange("p (i c) -> p i c", c=C)
    xb_3d = xb.rearrange("(i p) c -> p i c", p=128)

    # weights fp32 -> fp8 e4m3 DoubleRow slab layout ([p, slab, col]:
    # channel c = slab*128 + p contracts against the matching xbT slab)
    # wfg: f in cols 0:32, g in cols 32:64 (shared-rhs packing: one DR
    # matmul per key group computes f^T and g^T together)
    wfg_e4 = const.tile([128, 2 * 2 * CQ], E4, tag="wfg_e4")
    wfg_3 = wfg_e4[:].rearrange("p (k j) -> p k j", k=2)
    wh_e4 = const.tile([128, 2 * C], E4, tag="wh_e4")
    wh_e4_3 = wh_e4[:].rearrange("p (k c) -> p k c", k=2)
    for k in range(2):
        nc.gpsimd.tensor_copy(wfg_3[:, k, 0:CQ], wf_f[:, k * CQ:(k + 1) * CQ])
        nc.gpsimd.tensor_copy(wfg_3[:, k, CQ:2 * CQ],
                              wg_f[:, k * CQ:(k + 1) * CQ])
    nc.gpsimd.tensor_copy(wh_e4[:], wh_f[:])

    # f/g biases + scales as [64,1] columns matching the packed fg psum:
    # rows 0:32 f ((x@wf)+bf), rows 32:64 g (A_SCH*((x@wg)+bg))
    fgb_col = const.tile([2 * CQ, 1], F32, tag="fgb_col")
    nc.sync.dma_start(fgb_col[0:CQ, :], _col_ap(bf, CQ))
    nc.sync.dma_start(fgb_col[CQ:2 * CQ, :], _col_ap(bg, CQ))
    nc.gpsimd.tensor_scalar(fgb_col[CQ:2 * CQ, :], fgb_col[CQ:2 * CQ, :],
                            A_SCH, None, ALU.mult)
    fgs_col = const.tile([2 * CQ, 1], F32, tag="fgs_col")
    nc.vector.memset(fgs_col[0:CQ, :], 1.0)
    nc.vector.memset(fgs_col[CQ:2 * CQ, :], A_SCH)
    # bh enters each h psum through a trailing 1-row bf16 matmul (ones
    # lhsT x bh rhs) so no per-chunk vector add is needed; the gamma
    # scale in the copy-out covers it too
    bh_row = const.tile([1, C], F32, tag="bh_row")
    nc.sync.dma_start(bh_row[:], bass.AP(tensor=bh.tensor, offset=bh.offset,
                                         ap=[[0, 1], [1, C]]))
    bh_row_b = const.tile([1, C], BF16, tag="bh_row_b")
    nc.vector.tensor_copy(bh_row_b[:], bh_row[:])
    ones1_b = const.tile([1, 128], BF16, tag="ones1_b")
    nc.vector.memset(ones1_b[:], 1.0)
    gamma_bc = const.tile([128, 1], F32, tag="gamma_bc")
    nc.sync.dma_start(gamma_bc[:], _bcast_ap(gamma, 128, 1))

    ident_f = const.tile([128, 128], F32, tag="ident_f")
    from concourse.masks import make_identity
    make_identity(nc, ident_f[:])
    ident_b = const.tile([128, 128], BF16, tag="ident_b")
    nc.gpsimd.tensor_copy(ident_b[:], ident_f[:])

    # small consts
    two_e4 = const.tile([1, 128], E4, tag="two_e4")
    nc.vector.memset(two_e4[:], F_AUG)
    zero_e4 = const.tile([1, 128], E4, tag="zero_e4")
    nc.vector.memset(zero_e4[:], 0.0)
    sigb_bc = const.tile([128, 1], F32, tag="sigb_bc")
    nc.vector.memset(sigb_bc[:], -C_SCH / A_SCH)

    def bcast_sb(src_tile, parts, free):
        """AP replicating src_tile[0:1, ...] across partitions (and cols)."""
        ap = src_tile[0:1, 0:1]
        pstep = 1 if parts == 1 else 0
        if free <= 128:
            shape = [[pstep, parts], [1, free]]
        else:
            shape = [[pstep, parts], [0, free // 128], [1, 128]]
        return bass.AP(tensor=ap.tensor, offset=ap.offset, ap=shape)

    # ---- big SBUF tensors --------------------------------------------------
    f2 = big.tile([17, NCH * 2 * 128], E4, tag="f2")             # [p,(m,sl,c)]
    g2 = big.tile([17, 2 * N], E4, tag="g2")                     # [p,(sl,n)]
    stage_fg = big.tile([2 * CQ, N], E4, tag="stage_fg")         # f:0-31 g:32-63
    h_aug = big.tile([128, NCH * HAUG], E4, tag="h_aug")
    g_nat = big.tile([128, NCH * CQ], BF16, tag="g_nat")
    gsqn = big.tile([128, NCH * CQ], BF16, tag="gsqn")
    fsq_s = big.tile([32, 512], BF16, tag="fsq_s")               # scratch
    facc = big.tile([32, 8], F32, tag="facc")
    m_col = big.tile([128, NCH], BF16, tag="m_col")
    gn_col = big.tile([128, NCH], F32, tag="gn_col")
    mneg = big.tile([128, NCH], BF16, tag="mneg")
    fs_smp = big.tile([2 * CQ, 128], E4, tag="fs_smp")
    trv8 = big.tile([1, 8], F32, tag="trv8")
    trv_bc = big.tile([128, 1], F32, tag="trv_bc")
    scale_bc = big.tile([128, 1], F32, tag="scale_bc")
    sig_col = big.tile([128, NCH], F32, tag="sig_col")
    t2_col = big.tile([128, NCH], F32, tag="t2_col")
    t1_col = big.tile([128, NCH], F32, tag="t1_col")
    t3_col = big.tile([128, NCH], F32, tag="t3_col")

    f2_4d = f2[:].rearrange("p (m sl c) -> p m sl c", sl=2, c=128)
    g2_3d = g2[:].rearrange("p (sl n) -> p sl n", sl=2)
    h_aug_3d = h_aug[:].rearrange("p (m c) -> p m c", c=HAUG)
    g_nat_3d = g_nat[:].rearrange("p (m c) -> p m c", c=CQ)

    # round-robin engine dispatch for copy-out distribution. GPSIMD cannot
    # read PSUM, so drains rotate between ACT and DVE only.
    _engines = [nc.scalar, nc.vector]
    _rr = [0]

    def rr_copy(out, in_):
        e = _engines[_rr[0] % 2]
        _rr[0] += 1
        if e is nc.scalar:
            nc.scalar.copy(out, in_)
        else:
            e.tensor_copy(out, in_)

    def rr_add(out, in_, bias_ap):
        e = _engines[_rr[0] % 2]
        _rr[0] += 1
        if e is nc.scalar:
            nc.scalar.activation(out, in_, AF.Identity, bias=bias_ap)
        else:
            e.tensor_scalar(out, in_, bias_ap, None, ALU.add)

    def rr_add_scale(out, in_, bias_ap, scaled_bias_ap, scale):
        """out = (in_ + bias) * scale."""
        e = _engines[_rr[0] % 2]
        _rr[0] += 1
        if e is nc.scalar:
            nc.scalar.activation(out, in_, AF.Identity, scale=scale,
                                 bias=scaled_bias_ap)
        else:
            e.tensor_scalar(out, in_, bias_ap, scale, ALU.add, ALU.mult)

    # ---- prologue: transposes, projections, sample-max (interleaved) ------
    # aug rows first (consts only): f2 slab0 row16 = F_AUG, slab1 row16 = 0;
    # g2 row16 = 0 in both slabs (slab0 is read as 0 by the sample-max
    # matmuls, then overwritten with the -M~ row)
    nc.sync.dma_start(f2_4d[16:17, :, 0, :], bcast_sb(two_e4, 1, N))
    nc.sync.dma_start(f2_4d[16:17, :, 1, :], bcast_sb(zero_e4, 1, N))
    nc.sync.dma_start(g2_3d[16:17, :, :], bcast_sb(zero_e4, 1, 2 * N))

    with tc.tile_pool(name="ps_t", bufs=2, space="PSUM") as ps_t, \
         tc.tile_pool(name="ps_w", bufs=5, space="PSUM") as ps_w, \
         tc.tile_pool(name="ps_sub", bufs=1, space="PSUM") as ps_sub:

        def emit_xT(mt):
            """Group mt: PE-transpose 8 [128,128] bf16 blocks, drain the
            psum straight to the e4 DR slabs (no bf16 intermediate, one
            chain stage less). The first groups' drains go to ACT, which
            is otherwise idle in the head while DVE is the early
            bottleneck."""
            for k in range(2):
                tp = ps_t.tile([128, 512], BF16, tag="tp", name=f"tp{mt}_{k}")
                for idx, i in enumerate(range(mt * 4, mt * 4 + 4)):
                    nc.tensor.transpose(
                        tp[:, idx * 128:(idx + 1) * 128],
                        xn16[:, i * C + k * 128: i * C + k * 128 + 128],
                        ident_b[:])
                if mt < 4:
                    nc.scalar.copy(xbT_e43[:, k, mt * 512:(mt + 1) * 512],
                                   tp[:])
                else:
                    nc.vector.tensor_copy(
                        xbT_e43[:, k, mt * 512:(mt + 1) * 512], tp[:])

        emit_xT(0)
        emit_xT(1)

        def emit_mops(c0, c1):
            sl = slice(c0, c1)
            nc.scalar.activation(sig_col[:, sl], gn_col[:, sl], AF.Sqrt,
                                 scale=scale_bc[:])
            nc.gpsimd.tensor_scalar(t1_col[:, sl], m_col[:, sl],
                                    SIG_CAP * A_SCH, None, ALU.add)
            nc.vector.tensor_tensor(t2_col[:, sl], sig_col[:, sl],
                                    t1_col[:, sl], op=ALU.min)
            nc.vector.tensor_tensor(t3_col[:, sl], m_col[:, sl],
                                    t2_col[:, sl], op=ALU.max)
            nc.gpsimd.tensor_scalar(mneg[:, sl], t3_col[:, sl],
                                    -1.0 / F_AUG,
                                    (C_SCH - M_MARGIN * A_SCH) / F_AUG,
                                    ALU.mult, ALU.add)

        def emit_mfold(c0, c1):
            w = c1 - c0
            ps_mt = ps_w.tile([w, 128], BF16, tag="w", name=f"psmt{c0}")
            nc.tensor.transpose(ps_mt[:], mneg[:, c0:c1], ident_b[:])
            mst = fin.tile([32, 128], E4, tag="mst", name=f"mst{c0}")
            nc.vector.tensor_copy(mst[0:w, :], ps_mt[:])
            nc.sync.dma_start(g2_3d[16:17, 0, c0 * 128:c1 * 128], mst[0:w, :])

        for mt in range(8):
            if mt + 2 < 8:
                emit_xT(mt + 2)
            # packed f^T|g^T: one DR matmul per group ([64, 512] psum)
            ps_fg = ps_w.tile([2 * CQ, 512], F32, tag="w", name=f"psfg{mt}")
            nc.tensor.matmul(ps_fg[:], lhsT=wfg_3[:, :, :],
                             rhs=xbT_e43[:, :, mt * 512:(mt + 1) * 512],
                             start=True, stop=True, perf_mode=DR)

            # copy-out with per-row bias+scale (f rows: +bf, g rows:
            # A_SCH*(g+bg)) into staging; DMAs below remap to slab layout
            nc.scalar.activation(stage_fg[:, mt * 512:(mt + 1) * 512],
                                 ps_fg[:], AF.Identity, scale=fgs_col[:],
                                 bias=fgb_col[:])

            # |f|^2 accumulation for tr(F^T F) (group 0 sample is enough
            # for this global scale estimate)
            if mt == 0:
                nc.scalar.activation(fsq_s[:], ps_fg[0:CQ, :], AF.Square,
                                     accum_out=facc[:, 0:1])
                # f sample columns duplicated at partitions 32:64 so the
                # natural-orientation sample matmuls (lhsT = stage g rows)
                # see matching base partitions
                nc.sync.dma_start(fs_smp[CQ:2 * CQ, :],
                                  stage_fg[0:CQ, 0:128])

            # slab remap DMAs once per 2 groups (stage -> f2/g2 layouts)
            if mt % 2 == 1:
                m0 = (mt - 1) * 4
                sl0 = slice((mt - 1) * 512, (mt + 1) * 512)
                nc.sync.dma_start(f2_4d[0:16, m0:m0 + 8, 0, :],
                                  stage_fg[0:16, sl0])
                nc.sync.dma_start(f2_4d[0:16, m0:m0 + 8, 1, :],
                                  stage_fg[16:32, sl0])
                nc.sync.dma_start(g2_3d[0:16, 0, sl0], stage_fg[32:48, sl0])
                nc.sync.dma_start(g2_3d[0:16, 1, sl0], stage_fg[48:64, sl0])

            # g natural (for |g_n|^2): one [128, 128] psum per group
            ps_gn = ps_w.tile([128, 4 * CQ], F32, tag="w", name=f"psgn{mt}")
            for j in range(4):
                m = mt * 4 + j
                nc.tensor.matmul(ps_gn[:, j * CQ:(j + 1) * CQ],
                                 lhsT=xbT_e43[:, :, m * 128:(m + 1) * 128],
                                 rhs=wfg_3[:, :, CQ:2 * CQ],
                                 start=True, stop=True, perf_mode=DR)
            nc.vector.tensor_copy(g_nat_3d[:, mt * 4:(mt + 1) * 4, :], ps_gn[:])

            # h (fp8 DR, + bh via a 1-row bf16 matmul) -> gamma-scaled
            # fp8 h_aug
            for j2 in range(2):
                ps_h = ps_w.tile([128, 2 * C], F32, tag="w",
                                 name=f"psh{mt}_{j2}")
                for jj in range(2):
                    m = mt * 4 + 2 * j2 + jj
                    nc.tensor.matmul(
                        ps_h[:, jj * C:(jj + 1) * C],
                        lhsT=xbT_e43[:, :, m * 128:(m + 1) * 128],
                        rhs=wh_e4_3[:, :, :],
                        start=True, stop=False, perf_mode=DR)
                    nc.tensor.matmul(
                        ps_h[:, jj * C:(jj + 1) * C],
                        lhsT=ones1_b[:], rhs=bh_row_b[:],
                        start=False, stop=True)
                m0 = mt * 4 + 2 * j2
                nc.scalar.activation(h_aug_3d[:, m0:m0 + 2, 0:C], ps_h[:],
                                     AF.Identity, scale=gamma_bc[:])

            # |g_n|^2 incrementally for this group (Pool square + DVE
            # inner-axis reduce) so only the tail remains after group 7
            gsl = slice(mt * 4 * CQ, (mt + 1) * 4 * CQ)
            nc.gpsimd.tensor_tensor(gsqn[:, gsl], g_nat[:, gsl],
                                    g_nat[:, gsl], op=ALU.mult)
            gsq_g = gsqn[:, gsl].rearrange("p (m c) -> p m c", c=CQ)
            nc.vector.tensor_reduce(gn_col[:, mt * 4:(mt + 1) * 4], gsq_g,
                                    mybir.AxisListType.X, ALU.add)

            # tr(F^T F) estimate from the first 7 groups: start the DRAM
            # broadcast roundtrip early so it is off the critical path
            if mt == 0:
                facc_b = big.tile([32, 8], BF16, tag="facc_b")
                nc.gpsimd.tensor_copy(facc_b[:, 0:1], facc[:, 0:1])
                ones32 = const.tile([32, 1], BF16, tag="ones32")
                nc.vector.memset(ones32[:], 1.0)
                ps_tr = ps_w.tile([1, 8], F32, tag="w", name="ps_tr")
                nc.tensor.matmul(ps_tr[:, 0:1], lhsT=ones32[:],
                                 rhs=facc_b[:, 0:1], start=True, stop=True)
                trv1 = big.tile([1, 1], F32, tag="trv1")
                nc.scalar.copy(trv1[:], ps_tr[:, 0:1])
                scr_trv = nc.dram_tensor("scr_trv", [1], F32,
                                         kind="Internal").ap()
                nc.sync.dma_start(scr_trv, trv1[:])
                nc.sync.dma_start(trv_bc[:], _bcast_ap(scr_trv, 128, 1))
                nc.vector.tensor_scalar(scale_bc[:], trv_bc[:],
                                        8.0 * SIG_SCALE2 * A_SCH
                                        * A_SCH / N, None, ALU.mult)



            # sample-max for this group's 4 query chunks, straight off the
            # just-drained stage (natural orientation, non-DR): no wait on
            # the slab-remap DMAs or the aug rows, so the M~ chain runs a
            # full pair earlier. stage g rows are A_SCH-scaled like g2, so
            # m_col stays in the same units.
            for j in range(2):
                qc0 = mt * 4 + 2 * j
                ps_ss = ps_sub.tile([128, 256], F32, tag="ss",
                                    name=f"ss{qc0}")
                for jj in range(2):
                    nc.tensor.matmul(
                        ps_ss[:, jj * 128:(jj + 1) * 128],
                        lhsT=stage_fg[CQ:2 * CQ,
                                      (qc0 + jj) * 128:(qc0 + jj + 1) * 128],
                        rhs=fs_smp[CQ:2 * CQ, :],
                        start=True, stop=True)
                red_in = ps_ss[:].rearrange("p (a c) -> p a c", c=128)
                nc.vector.tensor_reduce(m_col[:, qc0:qc0 + 2], red_in,
                                        mybir.AxisListType.X, ALU.max)
            # M~ chain for the current pair as soon as its gn/sample are
            # done (scale roundtrip is ready from mt~2); pair (0,1) joins
            # at mt==3
            if mt % 2 == 1 and mt >= 3:
                if mt == 3:
                    emit_mops(0, 8)
                    emit_mfold(0, 8)
                emit_mops(4 * (mt - 1), 4 * (mt + 1))
                emit_mfold(4 * (mt - 1), 4 * (mt + 1))

    # h_aug ones column
    nc.vector.memset(h_aug_3d[:, :, C:C + 1], 1.0)

    # ---- main attention loop ----------------------------------------------
    ps_s = ctx.enter_context(tc.tile_pool(name="ps_s", bufs=4, space="PSUM"))
    ps_o = ctx.enter_context(tc.tile_pool(name="ps_o", bufs=4, space="PSUM"))
    ob_3d = ob.rearrange("(k p) c -> p k c", p=128)

    LAG = 4  # o-matmuls trail the s/exp stream by this many pair-tiles
    o_tiles: dict = {}
    e_tiles: dict = {}

    def emit_o(p):
        nb2, t2 = p // 16, p % 16
        o_ps = o_tiles[nb2]
        e_3d = e_tiles.pop(p)[:].rearrange("p (sl n) -> p sl n", sl=2)
        for q in range(4):
            nc.tensor.matmul(
                o_ps[q][:], lhsT=e_3d[:, :, q * 128:(q + 1) * 128],
                rhs=h_aug_3d[:, 2 * t2:2 * t2 + 2, :],
                start=(t2 == 0), stop=(t2 == 15), perf_mode=DR)
        if t2 == 15:
            finalize(nb2)

    def finalize(nb2):
        o_ps = o_tiles.pop(nb2)
        res4 = outp.tile([128, 4 * C], F32, tag="res4", name=f"res4_{nb2}")
        for q in range(4):
            gch = nb2 * 4 + q
            rs = fin.tile([128, 1], F32, tag="rs", name=f"rs{nb2}_{q}")
            nc.vector.tensor_scalar(rs[:], o_ps[q][:, C:C + 1], EPS_ROWSUM,
                                    None, ALU.add)
            recip = fin.tile([128, 1], F32, tag="recip", name=f"rc{nb2}_{q}")
            nc.vector.reciprocal_approx_fast(recip[:], rs[:])
            res_sc = fin.tile([128, C], F32, tag="res_sc",
                              name=f"rsc{nb2}_{q}")
            nc.scalar.activation(res_sc[:], o_ps[q][:, 0:C], AF.Identity,
                                 scale=recip[:])
            nc.gpsimd.tensor_tensor(
                res4[:, q * C:(q + 1) * C], res_sc[:],
                xf_f32[:, gch * C:(gch + 1) * C], op=ALU.add)
        nc.sync.dma_start(
            ob_3d[:, nb2 * 4:(nb2 + 1) * 4, :],
            res4[:].rearrange("p (k c) -> p k c", c=C))

    for p in range(NB * 16):
        nb, t = p // 16, p % 16
        if p == 2:
            # x fp32 for the residual path: DMA engines are idle during
            # the main loop; block-0 finalize consumes chunk 0 much later
            for i4 in range(4):
                nc.sync.dma_start(xf_f32_3d[:, i4 * 8:(i4 + 1) * 8, :],
                                  xb_3d[:, i4 * 8:(i4 + 1) * 8, :])
        if t == 0:
            o_tiles[nb] = [
                ps_o.tile([128, HAUG], F32, tag="o", name=f"o_ps{nb}_{q}")
                for q in range(4)]
        g2_blk = g2_3d[:, :, nb * 512:(nb + 1) * 512]
        e_t = epool.tile([128, 1024], E5, tag="e", name=f"e{nb}_{t}")
        e_tiles[p] = e_t
        # ACT runs ~99% vs DVE ~93% in the main loop: shift one exp half
        # to DVE in every other block to even the pair out
        patt = "DD" if (t == 7 and nb % 2 == 0) else PATT[t]
        for hh, eng_c in enumerate(patt):
            s_ps = ps_s.tile([128, 512], F32, tag="s", name=f"s{nb}_{t}_{hh}")
            nc.tensor.matmul(s_ps[:], lhsT=f2_4d[:, 2 * t + hh, :, :],
                             rhs=g2_blk, start=True, stop=True, perf_mode=DR)
            sl = slice(hh * 512, (hh + 1) * 512)
            if eng_c == "A":
                # s'' holds A_SCH*(s - M~) + C_SCH; undo for the sigmoid
                nc.scalar.activation(e_t[:, sl], s_ps[:], AF.Sigmoid,
                                     scale=1.0 / A_SCH, bias=sigb_bc[:])
            else:
                eng = nc.vector if eng_c == "D" else nc.gpsimd
                eng.tensor_scalar(e_t[:, sl].bitcast(I8), s_ps[:],
                                  0.0, C_SCH, ALU.max, ALU.min)
        if p >= LAG:
            emit_o(p - LAG)
    for p in range(NB * 16 - LAG, NB * 16):
        emit_o(p)


_CACHE: dict = {}


def build():
    if "nc" in _CACHE:
        return _CACHE["nc"]
    nc = bacc.Bacc("TRN2", target_bir_lowering=False, debug=False,
                   num_devices=N_CORES)
    io = {
        "xb": nc.dram_tensor("xb", [N, C], F32, kind="ExternalInput").ap(),
        "x16": nc.dram_tensor("x16", [N, C], BF16, kind="ExternalInput").ap(),
        "wf": nc.dram_tensor("wf", [C, CQ], F32, kind="ExternalInput").ap(),
        "wg": nc.dram_tensor("wg", [C, CQ], F32, kind="ExternalInput").ap(),
        "wh": nc.dram_tensor("wh", [C, C], F32, kind="ExternalInput").ap(),
        "bf": nc.dram_tensor("bf", [CQ], F32, kind="ExternalInput").ap(),
        "bg": nc.dram_tensor("bg", [CQ], F32, kind="ExternalInput").ap(),
        "bh": nc.dram_tensor("bh", [C], F32, kind="ExternalInput").ap(),
        "gamma": nc.dram_tensor("gamma", [1], F32, kind="ExternalInput").ap(),
        "ob": nc.dram_tensor("ob", [N, C], F32, kind="ExternalOutput").ap(),
    }
    with tile.TileContext(nc) as tc:
        with ExitStack() as ctx:
            _emit(ctx, tc, io)
    nc.compile()
    _CACHE["nc"] = nc
    return nc


def kernel(x, kernel_f, kernel_g, kernel_h, bias_f, bias_g, bias_h, gamma):
    import ml_dtypes
    x = np.asarray(x, dtype=np.float32)
    x16 = x.astype(ml_dtypes.bfloat16)
    wf = np.ascontiguousarray(np.asarray(kernel_f, dtype=np.float32))
    wg = np.ascontiguousarray(np.asarray(kernel_g, dtype=np.float32))
    wh = np.ascontiguousarray(np.asarray(kernel_h, dtype=np.float32))
    bf = np.ascontiguousarray(np.asarray(bias_f, dtype=np.float32))
    bg = np.ascontiguousarray(np.asarray(bias_g, dtype=np.float32))
    bh = np.ascontiguousarray(np.asarray(bias_h, dtype=np.float32))
    gm = np.ascontiguousarray(np.asarray(gamma, dtype=np.float32).reshape(1))

    per_core = {
        "xb": [np.ascontiguousarray(x[b].reshape(N, C)) for b in range(N_CORES)],
        "x16": [np.ascontiguousarray(x16[b].reshape(N, C))
                for b in range(N_CORES)],
        "wf": [wf] * N_CORES, "wg": [wg] * N_CORES, "wh": [wh] * N_CORES,
        "bf": [bf] * N_CORES, "bg": [bg] * N_CORES, "bh": [bh] * N_CORES,
        "gamma": [gm] * N_CORES,
    }
    nc = build()
    in_maps = [{nm: per_core[nm][b] for nm in per_core} for b in range(N_CORES)]
    try:
        res = bass_utils.run_bass_kernel_spmd(
            nc, in_maps, core_ids=list(range(N_CORES)))
    except ModuleNotFoundError:
        os.environ["BASS_NEVER_TRACE"] = "1"
        res = bass_utils.run_bass_kernel_spmd(
            nc, in_maps, core_ids=list(range(N_CORES)))
    out = np.stack([res.results[b]["ob"] for b in range(N_CORES)], axis=0)
    return out.reshape(B, HH, WW, C).astype(np.float32)


if __name__ == "__main__":
    rng = np.random.default_rng(0)
    x = rng.standard_normal((B, HH, WW, C)).astype(np.float32)
    lim = np.sqrt(6.0 / (C + CQ))
    out = kernel(
        x,
        rng.uniform(-lim, lim, (C, CQ)).astype(np.float32),
        rng.uniform(-lim, lim, (C, CQ)).astype(np.float32),
        rng.uniform(-lim, lim, (C, C)).astype(np.float32),
        np.zeros(CQ, np.float32), np.zeros(CQ, np.float32),
        np.zeros(C, np.float32), np.zeros(1, np.float32),
    )
    print(out.shape, out.dtype)

